# revision 1
# baseline (speedup 1.0000x reference)
"""nn_AlignerOT distributed Trainium2 kernel (8 NeuronCores).

Per-token 1D entropic OT: 50 log-domain Sinkhorn iterations over per-token
[512,512] cost matrices cost = 300*(x_i - y_j)^2, then ot = mean_n(P)*D*SCALE
+ delta_ot and out = src @ ot.

Distribution: token axis (N=256) sharded 32/core across 8 cores; one AllReduce
of the [512,512] P-sum at the end; every core then computes its own output
shard with the replicated ot matrix.

Core algorithm trick: the cost matrix is never materialized. The logsumexp
argument g_j - 300*(x_i - y_j)^2 = [g_j - 300 y_j^2] + [600 y_j] x_i - 300 x_i^2
is rank-3 in (i,j), so each [128,512] tile of it is ONE K=9 TensorE matmul of
bf16 3-limb decompositions (full fp32-class accuracy, full PE speed). The
per-partition -300 x_i^2 term and the logsumexp max-shift fold into the
ScalarE exp bias. The max-shift itself uses the previous iteration's
logsumexp (a tight upper bound, validated offline); a real max-reduce is only
needed for the first pass over each state variable.
"""

import sys

sys.path.insert(0, "/opt/trn_rl_repo")

import numpy as np
import ml_dtypes

from concourse import bacc, tile, mybir
from concourse.bass_utils import run_bass_kernel_spmd

F32 = mybir.dt.float32
BF16 = mybir.dt.bfloat16

REG = 0.1
SCALE = 300.0
D = 512
NCORES = 8
NTOK = 32            # tokens per core
NTOT = NCORES * NTOK
ITERS = 50
RLA = float(REG * np.log(1.0 / D))   # reg * log(a_i); uniform marginals
LA = float(np.log(1.0 / D))


def _limbs3(a):
    """f32 -> three bf16 limbs summing to ~f32 precision."""
    a = np.asarray(a, np.float32)
    l0 = a.astype(ml_dtypes.bfloat16)
    r1 = a - l0.astype(np.float32)
    l1 = r1.astype(ml_dtypes.bfloat16)
    r2 = r1 - l1.astype(np.float32)
    l2 = r2.astype(ml_dtypes.bfloat16)
    return l0, l1, l2


def _lhsT_host(v):
    """[NTOK,512] f32 values -> [9,16384] bf16 rows [1,1,1,v0,v0,v0,v1,v1,v2]."""
    v0, v1, v2 = _limbs3(v.reshape(-1))
    ones = np.ones(NTOK * 512, ml_dtypes.bfloat16)
    return np.stack([ones, ones, ones, v0, v0, v0, v1, v1, v2])


def _rhs_host(alpha, beta):
    """alpha,beta [NTOK,512] f32 -> [9,16384] bf16 rows [a0,a1,a2,b0,b1,b2,b0,b1,b0]."""
    a0, a1, a2 = _limbs3(alpha.reshape(-1))
    b0, b1, b2 = _limbs3(beta.reshape(-1))
    return np.stack([a0, a1, a2, b0, b1, b2, b0, b1, b0])


def _build(iters=ITERS):
    nc = bacc.Bacc("TRN2", target_bir_lowering=False, debug=False, num_devices=NCORES)

    lhsT1_e = nc.dram_tensor("lhsT1", [9, NTOK * 512], BF16, kind="ExternalInput")
    lhsT2_e = nc.dram_tensor("lhsT2", [9, NTOK * 512], BF16, kind="ExternalInput")
    rhs1_e = nc.dram_tensor("rhs1i", [9, NTOK * 512], BF16, kind="ExternalInput")
    rhs2_e = nc.dram_tensor("rhs2i", [9, NTOK * 512], BF16, kind="ExternalInput")
    xT_e = nc.dram_tensor("xT", [D, NTOK], F32, kind="ExternalInput")
    delta_e = nc.dram_tensor("delta", [D, D], F32, kind="ExternalInput")
    out_e = nc.dram_tensor("out", [NTOK, D], F32, kind="ExternalOutput")

    with tile.TileContext(nc, num_cores=NCORES) as tc:
        with (
            tc.tile_pool(name="state", bufs=1) as st,
            tc.tile_pool(name="work", bufs=2) as wk,
            tc.tile_pool(name="psum", bufs=4, space="PSUM") as ps,
            tc.tile_pool(name="psum_out", bufs=1, space="PSUM") as pso,
            tc.tile_pool(name="dram", bufs=1, space="DRAM") as dr,
        ):
            lhsT = [st.tile([9, NTOK * 512], BF16, name=f"lhsT{p}") for p in range(2)]
            rhs = [st.tile([9, NTOK * 512], BF16, name=f"rhs{p}") for p in range(2)]
            sig = [st.tile([128, 128], F32, name=f"sig{p}") for p in range(2)]
            sigu = st.tile([128, 128], F32)
            Scol = [st.tile([128, 128], F32, name=f"Scol{p}") for p in range(2)]
            biasc = [st.tile([128, 128], F32, name=f"biasc{p}") for p in range(2)]
            Pacc = st.tile([128, 4 * D], F32)
            delta_sb = st.tile([128, 4 * D], F32)
            srcT = st.tile([128, 4 * NTOK], F32)
            ar_sb = st.tile([128, 4 * D], F32)
            out_sb = st.tile([NTOK, D], F32)

            nc.sync.dma_start(out=lhsT[0][:], in_=lhsT1_e.ap())
            nc.sync.dma_start(out=lhsT[1][:], in_=lhsT2_e.ap())
            nc.sync.dma_start(out=rhs[0][:], in_=rhs1_e.ap())
            nc.sync.dma_start(out=rhs[1][:], in_=rhs2_e.ap())
            for t in range(4):
                nc.sync.dma_start(out=srcT[:, t * NTOK : (t + 1) * NTOK],
                                  in_=xT_e.ap()[t * 128 : (t + 1) * 128, :])
                nc.sync.dma_start(out=delta_sb[:, t * D : (t + 1) * D],
                                  in_=delta_e.ap()[t * 128 : (t + 1) * 128, :])
            nc.vector.memset(Pacc[:], 0.0)

            def emit_pass(p, fresh):
                """One Sinkhorn half-iteration. p=0: f-update (reads rhs[0],
                writes alpha rows of rhs[1]); p=1: g-update (the reverse)."""
                q = 1 - p
                if not fresh:
                    # bias = -sigma/reg from the stale (previous-iteration) LSE
                    nc.vector.tensor_scalar_mul(biasc[p][:], sig[p][:], -1.0 / REG)
                for n in range(NTOK):
                    for t in range(4):
                        col = n * 4 + t
                        pt = ps.tile([128, 512], F32, tag="mm", name="pt")
                        nc.tensor.matmul(
                            pt[:],
                            lhsT[p][:, col * 128 : (col + 1) * 128],
                            rhs[p][:, n * 512 : (n + 1) * 512],
                            start=True, stop=True,
                        )
                        if fresh:
                            nc.vector.tensor_reduce(
                                sigu[:, col : col + 1], pt[:],
                                axis=mybir.AxisListType.X, op=mybir.AluOpType.max)
                            nc.vector.tensor_scalar_mul(
                                biasc[p][:, col : col + 1],
                                sigu[:, col : col + 1], -1.0 / REG)
                        dump = wk.tile([128, 512], F32, tag="dump", name="dump")
                        nc.scalar.activation(
                            dump[:], pt[:], mybir.ActivationFunctionType.Exp,
                            bias=biasc[p][:, col : col + 1], scale=1.0 / REG,
                            accum_out=Scol[p][:, col : col + 1])
                # sigma' = sigma_used + reg*ln(S);  alpha_other = r_const - sigma'
                lnS = wk.tile([128, 128], F32, tag="lnS", name="lnS")
                nc.scalar.activation(lnS[:], Scol[p][:], mybir.ActivationFunctionType.Ln)
                src_sig = sigu if fresh else sig[p]
                nc.vector.scalar_tensor_tensor(
                    out=sig[p][:], in0=lnS[:], scalar=REG, in1=src_sig[:],
                    op0=mybir.AluOpType.mult, op1=mybir.AluOpType.add)
                acm = wk.tile([128, 128], F32, tag="acm", name="acm")
                nc.vector.tensor_scalar(
                    out=acm[:], in0=sig[p][:], scalar1=-1.0, scalar2=RLA,
                    op0=mybir.AluOpType.mult, op1=mybir.AluOpType.add)
                # 3-limb split (col-major), transpose via DMA xbar, flatten into rhs[q]
                L0 = wk.tile([128, 128], BF16, tag="L0", name="L0")
                L1 = wk.tile([128, 128], BF16, tag="L1", name="L1")
                L2 = wk.tile([128, 128], BF16, tag="L2", name="L2")
                R1 = wk.tile([128, 128], F32, tag="R1", name="R1")
                R2 = wk.tile([128, 128], F32, tag="R2", name="R2")
                nc.vector.tensor_copy(L0[:], acm[:])
                nc.vector.tensor_tensor(R1[:], acm[:], L0[:], mybir.AluOpType.subtract)
                nc.vector.tensor_copy(L1[:], R1[:])
                nc.vector.tensor_tensor(R2[:], R1[:], L1[:], mybir.AluOpType.subtract)
                nc.vector.tensor_copy(L2[:], R2[:])
                for k, L in enumerate((L0, L1, L2)):
                    LT = wk.tile([128, 128], BF16, tag=f"LT{k}", name=f"LT{k}")
                    nc.sync.dma_start(out=LT[:], in_=L[:], transpose=True)
                    nc.sync.dma_start(out=rhs[q][k : k + 1, :], in_=LT[:])

            # iteration 0 and 1: fresh max for pass1 of both, pass2 of iter 0
            emit_pass(0, True)
            emit_pass(1, True)
            emit_pass(0, True)
            emit_pass(1, False)
            if iters > 2:
                with tc.For_i(2, iters):
                    emit_pass(0, False)
                    emit_pass(1, False)

            # final P accumulation: P = exp((psum1 + f_i - 300 x_i^2)/reg),
            # bias = la - sigma1/reg
            nc.vector.tensor_scalar(
                out=biasc[0][:], in0=sig[0][:], scalar1=-1.0 / REG, scalar2=LA,
                op0=mybir.AluOpType.mult, op1=mybir.AluOpType.add)
            for n in range(NTOK):
                for t in range(4):
                    col = n * 4 + t
                    pt = ps.tile([128, 512], F32, tag="mm", name="ptf")
                    nc.tensor.matmul(
                        pt[:],
                        lhsT[0][:, col * 128 : (col + 1) * 128],
                        rhs[0][:, n * 512 : (n + 1) * 512],
                        start=True, stop=True,
                    )
                    et = wk.tile([128, 512], F32, tag="dump", name="et")
                    nc.scalar.activation(
                        et[:], pt[:], mybir.ActivationFunctionType.Exp,
                        bias=biasc[0][:, col : col + 1], scale=1.0 / REG)
                    nc.vector.tensor_tensor(
                        Pacc[:, t * D : (t + 1) * D],
                        Pacc[:, t * D : (t + 1) * D], et[:], mybir.AluOpType.add)

            # AllReduce the P-sum across the 8 cores
            ccin = dr.tile([D, D], F32)
            ccout = dr.tile([D, D], F32, addr_space="Shared")
            for t in range(4):
                nc.sync.dma_start(out=ccin[:][t * 128 : (t + 1) * 128, :],
                                  in_=Pacc[:, t * D : (t + 1) * D])
            nc.gpsimd.collective_compute(
                "AllReduce", mybir.AluOpType.add,
                replica_groups=[list(range(NCORES))],
                ins=[ccin[:].opt()], outs=[ccout[:].opt()])
            for t in range(4):
                nc.sync.dma_start(out=ar_sb[:, t * D : (t + 1) * D],
                                  in_=ccout[:][t * 128 : (t + 1) * 128, :])
            # ot = ar * (D*SCALE/NTOT) + delta
            nc.vector.scalar_tensor_tensor(
                out=ar_sb[:], in0=ar_sb[:], scalar=float(D * SCALE / NTOT),
                in1=delta_sb[:], op0=mybir.AluOpType.mult, op1=mybir.AluOpType.add)
            # out = src @ ot   (fp32 matmuls, K=128 per i-tile)
            po = pso.tile([NTOK, D], F32)
            for t in range(4):
                nc.tensor.matmul(
                    po[:],
                    srcT[:, t * NTOK : (t + 1) * NTOK],
                    ar_sb[:, t * D : (t + 1) * D],
                    start=(t == 0), stop=(t == 3),
                )
            nc.vector.tensor_copy(out_sb[:], po[:])
            nc.sync.dma_start(out=out_e.ap(), in_=out_sb[:])

    nc.compile()
    return nc


def _host_inputs(X, Y, delta_ot):
    """Build the 8 per-core input maps from the full problem inputs."""
    src = np.ascontiguousarray(X.reshape(-1, D).astype(np.float32))
    tgt = np.ascontiguousarray(Y.reshape(-1, D).astype(np.float32))
    delta = np.ascontiguousarray(delta_ot.astype(np.float32))
    maps = []
    for c in range(NCORES):
        x = src[c * NTOK : (c + 1) * NTOK]
        y = tgt[c * NTOK : (c + 1) * NTOK]
        lhsT1 = _lhsT_host(x)
        lhsT2 = _lhsT_host(y)
        rhs1 = _rhs_host(-SCALE * y * y, 600.0 * y)     # alpha1(g=0), beta1
        rhs2 = _rhs_host(np.zeros_like(x), 600.0 * x)   # alpha2 overwritten on-chip
        maps.append({
            "lhsT1": np.ascontiguousarray(lhsT1).view(np.uint16),
            "lhsT2": np.ascontiguousarray(lhsT2).view(np.uint16),
            "rhs1i": np.ascontiguousarray(rhs1).view(np.uint16),
            "rhs2i": np.ascontiguousarray(rhs2).view(np.uint16),
            "xT": np.ascontiguousarray(x.T),
            "delta": delta,
        })
    return maps


_cache = {}


def _get_nc(iters=ITERS):
    if iters not in _cache:
        _cache[iters] = _build(iters)
    return _cache[iters]


def kernel(X, Y, delta_ot, _iters=ITERS, _trace=False):
    nc = _get_nc(_iters)
    maps = _host_inputs(np.asarray(X), np.asarray(Y), np.asarray(delta_ot))
    res = run_bass_kernel_spmd(nc, maps, list(range(NCORES)), trace=_trace)
    out = np.concatenate([res.results[c]["out"] for c in range(NCORES)], axis=0)
    B, S = 2, 128
    out = out.reshape(B, S, D).astype(np.float32)
    if _trace:
        return out, res
    return out


# revision 3
# speedup vs baseline: 1.2179x; 1.2179x over previous
"""nn_AlignerOT distributed Trainium2 kernel (8 NeuronCores).

Per-token 1D entropic OT: 50 log-domain Sinkhorn iterations over per-token
[512,512] cost matrices cost = 300*(x_i - y_j)^2, then ot = mean_n(P)*D*SCALE
+ delta_ot and out = src @ ot.

Distribution: token axis (N=256) sharded 32/core across 8 cores; one AllReduce
of the [512,512] P-sum at the end; every core then computes its own output
shard with the replicated ot matrix.

Core tricks:
- The cost matrix is never materialized. The logsumexp argument
  g_j - 300(x_i-y_j)^2 - sigma_i = [g_j - 300 y_j^2] + [600 y_j] x_i
  + [-300 x_i^2 - sigma_i] is rank-3 in (i,j) (the -300 x_i^2 cancels
  against the alpha definition; see below), so each [128,512] tile of it is
  ONE K=12 TensorE matmul of bf16 3-limb decompositions (fp32-class
  accuracy at full PE speed).
- The logsumexp max-shift sigma is the previous iteration's logsumexp
  (a tight upper bound, validated offline: per-iteration |dg| <= 0.23 after
  iteration 1). A real max-reduce is only needed for 3 of the 100 passes.
- With sigma inside the matmul, the exp needs no per-partition bias, so one
  ScalarE instruction exponentiates a whole token (FD=2048 across 4 PSUM
  banks), amortizing the ~350-cycle ACT instruction overhead.
- Row sums of exp come from DVE tensor_reduce over the bf16 exp dump, with a
  few tokens left on the ACT accum_out path to balance ACT vs DVE time.
"""

import sys

sys.path.insert(0, "/opt/trn_rl_repo")

import numpy as np
import ml_dtypes

from concourse import bacc, tile, mybir
from concourse import hw_specs
from concourse.bass_utils import run_bass_kernel_spmd

F32 = mybir.dt.float32
BF16 = mybir.dt.bfloat16

REG = 0.1
SCALE = 300.0
D = 512
NCORES = 8
NTOK = 32            # tokens per core
NTOT = NCORES * NTOK
ITERS = 50
NACC = 3             # tokens per pass summed via ACT accum_out (engine balance)
RLA = float(REG * np.log(1.0 / D))   # reg * log(a_i); uniform marginals
LA = float(np.log(1.0 / D))

# Force every activation onto the one table set that holds both Exp and Ln,
# so the compiler hoists a single ACT_TABLE_LOAD instead of thrashing
# exp_and_others <-> natural_log every pass. Indices into act_info.json must
# be preserved, so empty the other sets rather than removing them.
_orig_get_tables = hw_specs.get_activation_tables


def _patched_tables(arch):
    t = _orig_get_tables(arch)
    keep = "natural_log_exp_and_others"
    if keep in t:
        t = {k: (v if k == keep else set()) for k, v in t.items()}
    return t


hw_specs.get_activation_tables = _patched_tables
bacc.get_activation_tables = _patched_tables


def _limbs3(a):
    """f32 -> three bf16 limbs summing to ~f32 precision."""
    a = np.asarray(a, np.float32)
    l0 = a.astype(ml_dtypes.bfloat16)
    r1 = a - l0.astype(np.float32)
    l1 = r1.astype(ml_dtypes.bfloat16)
    r2 = r1 - l1.astype(np.float32)
    l2 = r2.astype(ml_dtypes.bfloat16)
    return l0, l1, l2


def _lhsT_host(v):
    """[NTOK,512] f32 -> [12,16384] bf16 rows [1,1,1,v0,v0,v0,v1,v1,v2,0,0,0].

    Rows 9-11 are the per-iteration sigma limbs (start at zero)."""
    v0, v1, v2 = _limbs3(v.reshape(-1))
    ones = np.ones(NTOK * 512, ml_dtypes.bfloat16)
    zero = np.zeros(NTOK * 512, ml_dtypes.bfloat16)
    return np.stack([ones, ones, ones, v0, v0, v0, v1, v1, v2, zero, zero, zero])


def _rhs_host(alpha, beta):
    """[12,16384] bf16 rows [a0,a1,a2,b0,b1,b2,b0,b1,b0,-1,-1,-1].

    Rows 9-11 multiply the lhsT sigma limbs: psum gets -sigma_i."""
    a0, a1, a2 = _limbs3(alpha.reshape(-1))
    b0, b1, b2 = _limbs3(beta.reshape(-1))
    mone = np.full(NTOK * 512, -1.0, ml_dtypes.bfloat16)
    return np.stack([a0, a1, a2, b0, b1, b2, b0, b1, b0, mone, mone, mone])


def _build(iters=ITERS):
    nc = bacc.Bacc("TRN2", target_bir_lowering=False, debug=False, num_devices=NCORES)

    lhsT1_e = nc.dram_tensor("lhsT1", [12, NTOK * 512], BF16, kind="ExternalInput")
    lhsT2_e = nc.dram_tensor("lhsT2", [12, NTOK * 512], BF16, kind="ExternalInput")
    rhs1_e = nc.dram_tensor("rhs1i", [12, NTOK * 512], BF16, kind="ExternalInput")
    rhs2_e = nc.dram_tensor("rhs2i", [12, NTOK * 512], BF16, kind="ExternalInput")
    xT_e = nc.dram_tensor("xT", [D, NTOK], F32, kind="ExternalInput")
    delta_e = nc.dram_tensor("delta", [D, D], F32, kind="ExternalInput")
    out_e = nc.dram_tensor("out", [NTOK, D], F32, kind="ExternalOutput")

    with tile.TileContext(nc, num_cores=NCORES) as tc:
        with (
            tc.tile_pool(name="state", bufs=1) as st,
            tc.tile_pool(name="work", bufs=2) as wk,
            tc.tile_pool(name="dumps", bufs=4) as dp,
            tc.tile_pool(name="psum", bufs=2, space="PSUM") as ps,
            tc.tile_pool(name="dram", bufs=1, space="DRAM") as dr,
        ):
            lhsT = [st.tile([12, NTOK * 512], BF16, name=f"lhsT{p}") for p in range(2)]
            rhs = [st.tile([12, NTOK * 512], BF16, name=f"rhs{p}") for p in range(2)]
            sig = [st.tile([128, 128], F32, name=f"sig{p}") for p in range(2)]
            sigu = st.tile([128, 128], F32)
            biasc = st.tile([128, 128], F32)
            Scol = [st.tile([128, 128], F32, name=f"Scol{p}") for p in range(2)]
            Pacc = st.tile([128, 4 * D], F32)
            delta_sb = st.tile([128, 4 * D], F32)
            srcT = st.tile([128, 4 * NTOK], F32)
            ar_sb = st.tile([128, 4 * D], F32)
            out_sb = st.tile([NTOK, D], F32)

            nc.sync.dma_start(out=lhsT[0][:], in_=lhsT1_e.ap())
            nc.sync.dma_start(out=lhsT[1][:], in_=lhsT2_e.ap())
            nc.sync.dma_start(out=rhs[0][:], in_=rhs1_e.ap())
            nc.sync.dma_start(out=rhs[1][:], in_=rhs2_e.ap())
            for t in range(4):
                nc.sync.dma_start(out=srcT[:, t * NTOK : (t + 1) * NTOK],
                                  in_=xT_e.ap()[t * 128 : (t + 1) * 128, :])
                nc.sync.dma_start(out=delta_sb[:, t * D : (t + 1) * D],
                                  in_=delta_e.ap()[t * 128 : (t + 1) * 128, :])
            la_bias = st.tile([128, 1], F32)
            nc.vector.memset(la_bias[:], LA)
            nc.vector.memset(Pacc[:], 0.0)
            nc.vector.memset(sig[0][:], 0.0)
            nc.vector.memset(sig[1][:], 0.0)

            def emit_smalls(p, fresh):
                """sigma' = sigma_in + [fresh max] + reg*ln(S); update the sigma
                limbs of lhsT[p] and the alpha limbs of rhs[1-p]."""
                q = 1 - p
                lnS = wk.tile([128, 128], F32, tag="lnS", name="lnS")
                nc.scalar.activation(lnS[:], Scol[p][:], mybir.ActivationFunctionType.Ln)
                if fresh:
                    tmp = wk.tile([128, 128], F32, tag="tmp", name="tmp")
                    nc.vector.scalar_tensor_tensor(
                        out=tmp[:], in0=lnS[:], scalar=REG, in1=sigu[:],
                        op0=mybir.AluOpType.mult, op1=mybir.AluOpType.add)
                    nc.vector.tensor_tensor(sig[p][:], tmp[:], sig[p][:],
                                            mybir.AluOpType.add)
                else:
                    nc.vector.scalar_tensor_tensor(
                        out=sig[p][:], in0=lnS[:], scalar=REG, in1=sig[p][:],
                        op0=mybir.AluOpType.mult, op1=mybir.AluOpType.add)
                # alpha_other = RLA - sigma  (col-major)
                acm = wk.tile([128, 128], F32, tag="acm", name="acm")
                nc.vector.tensor_scalar(
                    out=acm[:], in0=sig[p][:], scalar1=-1.0, scalar2=RLA,
                    op0=mybir.AluOpType.mult, op1=mybir.AluOpType.add)
                # 3-limb split of alpha -> rhs[q] rows 0-2, and of sigma ->
                # lhsT[p] rows 9-11, via DMA xbar transpose + flatten.
                for src_cm, dst, base in ((acm, rhs[q], 0), (sig[p], lhsT[p], 9)):
                    L0 = wk.tile([128, 128], BF16, tag="L0", name="L0")
                    L1 = wk.tile([128, 128], BF16, tag="L1", name="L1")
                    L2 = wk.tile([128, 128], BF16, tag="L2", name="L2")
                    R1 = wk.tile([128, 128], F32, tag="R1", name="R1")
                    R2 = wk.tile([128, 128], F32, tag="R2", name="R2")
                    nc.vector.tensor_copy(L0[:], src_cm[:])
                    nc.vector.tensor_tensor(R1[:], src_cm[:], L0[:], mybir.AluOpType.subtract)
                    nc.vector.tensor_copy(L1[:], R1[:])
                    nc.vector.tensor_tensor(R2[:], R1[:], L1[:], mybir.AluOpType.subtract)
                    nc.vector.tensor_copy(L2[:], R2[:])
                    for k, L in enumerate((L0, L1, L2)):
                        LT = wk.tile([128, 128], BF16, tag=f"LT{k}", name=f"LT{k}")
                        nc.sync.dma_start(out=LT[:], in_=L[:], transpose=True)
                        nc.sync.dma_start(out=dst[base + k : base + k + 1, :], in_=LT[:])

            def emit_pass_fresh(p):
                """Peeled pass: per-tile FD=512 exp with DVE max + AP bias +
                ACT accum (sigma rows of lhsT may hold a stale shift; the max
                is over the shifted psum, so sigma' = sigma_in + max + reg lnS)."""
                for n in range(NTOK):
                    pt = ps.tile([128, 2048], F32, tag="mm", name="pt")
                    for t in range(4):
                        col = n * 4 + t
                        nc.tensor.matmul(
                            pt[:, t * 512 : (t + 1) * 512],
                            lhsT[p][:, col * 128 : (col + 1) * 128],
                            rhs[p][:, n * 512 : (n + 1) * 512],
                            start=True, stop=True)
                        nc.vector.tensor_reduce(
                            sigu[:, col : col + 1], pt[:, t * 512 : (t + 1) * 512],
                            axis=mybir.AxisListType.X, op=mybir.AluOpType.max)
                        nc.vector.tensor_scalar_mul(
                            biasc[:, col : col + 1], sigu[:, col : col + 1], -1.0 / REG)
                        dump = dp.tile([128, 512], BF16, tag="dumpf", name="dumpf")
                        nc.scalar.activation(
                            dump[:], pt[:, t * 512 : (t + 1) * 512],
                            mybir.ActivationFunctionType.Exp,
                            bias=biasc[:, col : col + 1], scale=1.0 / REG,
                            accum_out=Scol[p][:, col : col + 1])
                emit_smalls(p, fresh=True)

            def emit_pass(p):
                """Steady-state pass: sigma shift inside the matmul, one
                FD=2048 exp per token; sums on DVE except NACC tokens on ACT."""
                for n in range(NTOK):
                    pt = ps.tile([128, 2048], F32, tag="mm", name="pt")
                    for t in range(4):
                        col = n * 4 + t
                        nc.tensor.matmul(
                            pt[:, t * 512 : (t + 1) * 512],
                            lhsT[p][:, col * 128 : (col + 1) * 128],
                            rhs[p][:, n * 512 : (n + 1) * 512],
                            start=True, stop=True)
                    if n < NACC:
                        for t in range(4):
                            col = n * 4 + t
                            dump = dp.tile([128, 512], BF16, tag="dumpf", name="dumpf")
                            nc.scalar.activation(
                                dump[:], pt[:, t * 512 : (t + 1) * 512],
                                mybir.ActivationFunctionType.Exp,
                                scale=1.0 / REG,
                                accum_out=Scol[p][:, col : col + 1])
                    else:
                        dump = dp.tile([128, 2048], BF16, tag="dump", name="dump")
                        nc.scalar.activation(
                            dump[:], pt[:], mybir.ActivationFunctionType.Exp,
                            scale=1.0 / REG)
                        for t in range(4):
                            col = n * 4 + t
                            nc.vector.tensor_reduce(
                                Scol[p][:, col : col + 1],
                                dump[:, t * 512 : (t + 1) * 512],
                                axis=mybir.AxisListType.X, op=mybir.AluOpType.add)
                emit_smalls(p, fresh=False)

            # iterations 0,1 peeled: fresh max for pass1 of both and pass2 of 0
            emit_pass_fresh(0)
            emit_pass_fresh(1)
            emit_pass_fresh(0)
            emit_pass(1)
            if iters > 2:
                with tc.For_i(2, iters):
                    emit_pass(0)
                    emit_pass(1)

            # final P accumulation: with sigma1 in the matmul,
            # (f_i + g_j - c_ij)/reg = psum/reg + log(1/D) exactly.
            for n in range(NTOK):
                pt = ps.tile([128, 2048], F32, tag="mm", name="ptf")
                for t in range(4):
                    col = n * 4 + t
                    nc.tensor.matmul(
                        pt[:, t * 512 : (t + 1) * 512],
                        lhsT[0][:, col * 128 : (col + 1) * 128],
                        rhs[0][:, n * 512 : (n + 1) * 512],
                        start=True, stop=True)
                et = dp.tile([128, 2048], BF16, tag="dump", name="et")
                nc.scalar.activation(et[:], pt[:], mybir.ActivationFunctionType.Exp,
                                     bias=la_bias[:], scale=1.0 / REG)
                nc.vector.tensor_tensor(Pacc[:], Pacc[:], et[:], mybir.AluOpType.add)

            # AllReduce the P-sum across the 8 cores
            ccin = dr.tile([D, D], F32)
            ccout = dr.tile([D, D], F32, addr_space="Shared")
            for t in range(4):
                nc.sync.dma_start(out=ccin[:][t * 128 : (t + 1) * 128, :],
                                  in_=Pacc[:, t * D : (t + 1) * D])
            nc.gpsimd.collective_compute(
                "AllReduce", mybir.AluOpType.add,
                replica_groups=[list(range(NCORES))],
                ins=[ccin[:].opt()], outs=[ccout[:].opt()])
            for t in range(4):
                nc.sync.dma_start(out=ar_sb[:, t * D : (t + 1) * D],
                                  in_=ccout[:][t * 128 : (t + 1) * 128, :])
            # ot = ar * (D*SCALE/NTOT) + delta
            nc.vector.scalar_tensor_tensor(
                out=ar_sb[:], in0=ar_sb[:], scalar=float(D * SCALE / NTOT),
                in1=delta_sb[:], op0=mybir.AluOpType.mult, op1=mybir.AluOpType.add)
            # out = src @ ot   (fp32 matmuls, K=128 per i-tile)
            po = ps.tile([128, 2048], F32, tag="mm", name="po")
            for t in range(4):
                nc.tensor.matmul(
                    po[0:NTOK, 0:D],
                    srcT[:, t * NTOK : (t + 1) * NTOK],
                    ar_sb[:, t * D : (t + 1) * D],
                    start=(t == 0), stop=(t == 3))
            nc.vector.tensor_copy(out_sb[:], po[0:NTOK, 0:D])
            nc.sync.dma_start(out=out_e.ap(), in_=out_sb[:])

    nc.compile()
    return nc


def _host_inputs(X, Y, delta_ot):
    """Build the 8 per-core input maps from the full problem inputs."""
    src = np.ascontiguousarray(X.reshape(-1, D).astype(np.float32))
    tgt = np.ascontiguousarray(Y.reshape(-1, D).astype(np.float32))
    delta = np.ascontiguousarray(delta_ot.astype(np.float32))
    maps = []
    for c in range(NCORES):
        x = src[c * NTOK : (c + 1) * NTOK]
        y = tgt[c * NTOK : (c + 1) * NTOK]
        maps.append({
            "lhsT1": np.ascontiguousarray(_lhsT_host(x)).view(np.uint16),
            "lhsT2": np.ascontiguousarray(_lhsT_host(y)).view(np.uint16),
            "rhs1i": np.ascontiguousarray(_rhs_host(-SCALE * y * y, 600.0 * y)).view(np.uint16),
            "rhs2i": np.ascontiguousarray(_rhs_host(np.zeros_like(x), 600.0 * x)).view(np.uint16),
            "xT": np.ascontiguousarray(x.T),
            "delta": delta,
        })
    return maps


_cache = {}


def _get_nc(iters=ITERS):
    if iters not in _cache:
        _cache[iters] = _build(iters)
    return _cache[iters]


def kernel(X, Y, delta_ot, _iters=ITERS, _trace=False):
    nc = _get_nc(_iters)
    maps = _host_inputs(np.asarray(X), np.asarray(Y), np.asarray(delta_ot))
    res = run_bass_kernel_spmd(nc, maps, list(range(NCORES)), trace=_trace)
    out = np.concatenate([res.results[c]["out"] for c in range(NCORES)], axis=0)
    B, S = 2, 128
    out = out.reshape(B, S, D).astype(np.float32)
    if _trace:
        return out, res
    return out
